# revision 31
# baseline (speedup 1.0000x reference)
"""Multi-head self-attention (B=4, S=2048, D=2048, H=16, hd=128) on 8 trn2
NeuronCores.

Sharding: tensor-parallel over heads. Core c owns heads {2c, 2c+1}:
  - computes q/k/v projections for its 2 heads over all tokens,
  - runs causal attention for its (4 batches x 2 heads) units,
  - computes a partial output projection with its 256 rows of Wo.
Host sums the 8 partial (bf16) outputs and adds bo.

Single woven PE stream per batch (c-outer):
  proj chunk c -> attention units (c,h0),(c,h1) -> [outproj of chunk c-1
  interleaved as PE filler inside the ACT-paced unit pair loops].

Layouts:
  qT/kT: [j, t] from W-stationary matmuls (lhsT = W tile, rhs = xT tile).
  v:     [t, j] DIRECTLY from x-stationary matmuls (lhsT = xT tile,
         rhs = Wv tile) -- no PE transposes. Wv is augmented per head with
         a 129th column of zeros whose bias is 1.0, so each v key-tile
         slab [t, 129] carries an all-ones column.
  S^T:   [t_k, t_q] score tiles (lhsT = kT tile, rhs = qT chunk); exp on
         ACT (no max subtraction -- weights are 0.02-scaled, logits O(1));
         diagonal tiles get 0/1 masks multiplied after exp (on GPSIMD).
  U:     [t_q(128-block), 129] = accumulated (lhsT = e block, rhs = v slab)
         -- column 128 is the softmax denominator G for free (ones column
         of v). Normalize: DVE reciprocal of G [128,1], then ACT
         Copy-with-scale evacuates U*(1/G) -> yn [q, j] bf16.
  ynT:   one XBAR DMA transpose per unit turns yn [q, (blk, j)] into
         ynT [j, (blk, q)] -- no PSUM, no PE, no DVE.
  O^T:   [d', t] partial = (lhsT = Wo tile [j, d'], rhs = ynT chunk),
         stored to DRAM as bf16 (host sums partials in f32).

PSUM banks: s2 pairs 2x2 + U 2 + pp (proj/outproj chains) 2 = 8.
"""

import math
from collections import deque

import numpy as np
import ml_dtypes

import concourse.bass as bass
import concourse.bacc as bacc
import concourse.mybir as mybir
import concourse.tile as tile
from concourse.bass_utils import run_bass_kernel_spmd

BF16 = mybir.dt.bfloat16
F32 = mybir.dt.float32

B, S, D_MODEL = 4, 2048, 2048
N_HEADS, HEAD_DIM = 16, 128
N_CORES = 8
H_PER = N_HEADS // N_CORES          # 2 heads per core
JL = H_PER * HEAD_DIM               # 256 local j-columns
JV = H_PER * (HEAD_DIM + 1)         # 258 v-columns incl. per-head ones col
T = B * S                           # 8192 tokens
KD = D_MODEL // 128                 # 16 contraction tiles over d_model
TC = S // 512                       # 4 token chunks of 512 per batch
NQK = 2 * H_PER                     # 4 j-tiles for fused q+k
SCALE = 1.0 / math.sqrt(HEAD_DIM)

_CACHED_NC = None

# schedule-tuning knobs (sweepable)
E_BUFS = 9        # e-pair SBUF ring depth
POP_PAIR = 1      # filler pops per score pair
POP_AFTER = 3     # filler pops after the pair loop
POP_BLK = 1       # filler pops per U block
OSB_BUFS = 8      # o_sb ring depth
YN_BUFS = 2
GI_BUFS = 4
OCOPY_MOD = 1000  # 1-in-OCOPY_MOD o_sb copies go to ACT (1000 = all DVE)
NORM_ENGINE = "act"  # U*(1/G) evacuation engine: act | dve
COPY_MIX_C0 = False  # alternate o_sb copies DVE/ACT in c0 units


def build_program():
    nc = bacc.Bacc("TRN2", target_bir_lowering=False, debug=False)

    xT = nc.dram_tensor("xT", [D_MODEL, T], BF16, kind="ExternalInput").ap()
    wqk = nc.dram_tensor("wqk", [D_MODEL, NQK * 128], BF16, kind="ExternalInput").ap()
    bqk = nc.dram_tensor("bqk", [NQK * 128], F32, kind="ExternalInput").ap()
    wv = nc.dram_tensor("wv", [D_MODEL, JV], BF16, kind="ExternalInput").ap()
    bvb = nc.dram_tensor("bvb", [128, JV], F32, kind="ExternalInput").ap()
    wo = nc.dram_tensor("wo", [JL, D_MODEL], BF16, kind="ExternalInput").ap()
    outT = nc.dram_tensor("outT", [D_MODEL, T], BF16, kind="ExternalOutput").ap()

    xT_r = xT.rearrange("(k p) t -> p k t", p=128)        # [128, KD, T]

    with tile.TileContext(nc) as tc:
        with (
            tc.tile_pool(name="const", bufs=1) as const,
            tc.tile_pool(name="work", bufs=1) as work,
            tc.tile_pool(name="psum", bufs=1, space="PSUM") as psum,
        ):
            # ---- constants: only wqk k-chunk 0 + bqk lead the first xt ----
            wqk_sb = const.tile([128, KD, NQK * 128], BF16)
            wqk_r = wqk.rearrange("(k p) j -> p k j", p=128)
            nc.sync.dma_start(wqk_sb[:, 0:4, :], wqk_r[:, 0:4, :])
            bqk_sb = const.tile([128, NQK], F32)
            nc.sync.dma_start(bqk_sb[:], bqk.rearrange("(m p) -> p m", p=128))

            wv_sb = const.tile([128, KD, JV], BF16)
            bvb_sb = const.tile([128, JV], F32)
            wo_sb = const.tile([128, JL // 128, D_MODEL], BF16)

            def load_seg_consts(seg):
                # trailing consts after the last xt segment, in need order
                if seg < 3:
                    return
                wv_r = wv.rearrange("(k p) j -> p k j", p=128)
                for kc in range(1, 4):
                    nc.sync.dma_start(wqk_sb[:, 4 * kc:4 * (kc + 1), :],
                                      wqk_r[:, 4 * kc:4 * (kc + 1), :])
                    nc.sync.dma_start(wv_sb[:, 4 * (kc - 1):4 * kc, :],
                                      wv_r[:, 4 * (kc - 1):4 * kc, :])
                nc.sync.dma_start(wv_sb[:, 12:16, :], wv_r[:, 12:16, :])
                nc.sync.dma_start(bvb_sb[:], bvb[:, :])
                nc.sync.dma_start(
                    wo_sb[:], wo.rearrange("(k p) d -> p k d", p=128))

            # masks[i][r, u] = 1.0 if u >= 128*i + r else 0  (diagonal tiles)
            masks = const.tile([128, 4, 512], BF16)
            nc.gpsimd.memset(masks[:], 1.0)
            for i in range(4):
                nc.gpsimd.affine_select(
                    out=masks[:, i, :],
                    in_=masks[:, i, :],
                    compare_op=mybir.AluOpType.is_ge,
                    fill=0.0,
                    base=-128 * i,
                    pattern=[[1, 512]],
                    channel_multiplier=-1,
                )

            st = _State()
            for b in range(B):
                _emit_batch(nc, tc, work, psum, b,
                            xT_r, wqk_sb, bqk_sb, wv_sb, bvb_sb, wo_sb,
                            masks, outT, st,
                            post_first_xt=load_seg_consts if b == 0 else None)
            st.copy_mix = True
            while st.filler:
                st.filler.popleft()()

    nc.compile()
    return nc


class _State:
    def __init__(self):
        self.filler = deque()      # outproj-group thunks, ready to emit
        self.ocopy_i = 0           # o_sb evacuation round-robin counter
        self.copy_mix = False      # True where ACT has slack (c0 units, drain)


def _emit_batch(nc, tc, work, psum, b, xT_r, wqk_sb, bqk_sb, wv_sb, bvb_sb,
                wo_sb, masks, outT, st, post_first_xt=None):
    t0 = b * S

    qkT = work.tile([128, NQK, S], BF16, tag="qkT", bufs=2)
    v_sb = work.tile([128, S // 128, H_PER, HEAD_DIM + 1], BF16, tag="v", bufs=1)
    ynT = work.tile([128, H_PER, S], BF16, tag="ynT", bufs=2)

    def pop(n):
        for _ in range(n):
            if st.filler:
                st.filler.popleft()()

    for c in range(TC):
        # ---------------- projection chunk c ----------------
        with nc.named_scope(f"proj.b{b}.c{c}"):
            xt = work.tile([128, KD, 512], BF16, tag="xt", bufs=2)
            for seg in range(4):
                nc.sync.dma_start(
                    xt[:, 4 * seg:4 * (seg + 1), :],
                    xT_r[:, 4 * seg:4 * (seg + 1), t0 + c * 512: t0 + (c + 1) * 512])
                if post_first_xt is not None:
                    post_first_xt(seg)
            post_first_xt = None
            # q/k chains (W-stationary): qkT[j, t]
            def qk_bias(jm, ps):
                nc.vector.tensor_scalar_add(
                    qkT[:, jm, c * 512:(c + 1) * 512], ps[:],
                    bqk_sb[:, jm:jm + 1],
                )

            if b == 0 and c == 0:
                # first chunk: ping-pong two chains across the arriving
                # wqk/xt k-segments so PE starts before all DMAs land
                ps01 = [psum.tile([128, 512], F32, tag="pp", bufs=2,
                                  name=f"ps0{i}")
                        for i in range(2)]
                for kc in range(4):
                    for jm in range(2):
                        for k in range(4 * kc, 4 * kc + 4):
                            nc.tensor.matmul(
                                ps01[jm][:],
                                lhsT=wqk_sb[:, k, jm * 128:(jm + 1) * 128],
                                rhs=xt[:, k, :],
                                start=(k == 0), stop=(k == KD - 1),
                            )
                for jm in range(2):
                    qk_bias(jm, ps01[jm])
                jm_rest = range(2, NQK)
            else:
                jm_rest = range(NQK)
            for jm in jm_rest:
                ps = psum.tile([128, 512], F32, tag="pp", bufs=2)
                for k in range(KD):
                    nc.tensor.matmul(
                        ps[:],
                        lhsT=wqk_sb[:, k, jm * 128:(jm + 1) * 128],
                        rhs=xt[:, k, :],
                        start=(k == 0), stop=(k == KD - 1),
                    )
                qk_bias(jm, ps)
            # v chains (x-stationary): v[t, j] slabs with ones columns
            for blk in range(4):
                m = 4 * c + blk
                pv = psum.tile([128, JV], F32, tag="pp", bufs=2)
                for k in range(KD):
                    nc.tensor.matmul(
                        pv[:],
                        lhsT=xt[:, k, blk * 128:(blk + 1) * 128],
                        rhs=wv_sb[:, k, :],
                        start=(k == 0), stop=(k == KD - 1),
                    )
                nc.vector.tensor_add(
                    v_sb[:, m, :, :],
                    pv.rearrange("p (h j) -> p h j", h=H_PER),
                    bvb_sb.rearrange("p (h j) -> p h j", h=H_PER),
                )

        # ---------------- attention units (c, h) ----------------
        nm = 4 * (c + 1)            # valid 128-wide key tiles
        npr = nm // 2
        st.copy_mix = COPY_MIX_C0 and (c == 0)
        for h in range(H_PER):
          with nc.named_scope(f"att.b{b}.c{c}.h{h}"):
            e_pairs = []
            for pr in range(npr):
                pop(POP_PAIR)
                s2 = psum.tile([128, 2, 512], F32, tag="s2", bufs=2)
                for i in range(2):
                    m = 2 * pr + i
                    # causal: tile m only sees queries q >= 128*(m-4c);
                    # the unwritten psum region reads 0 (start zeroes the
                    # bank) and the mask multiply zeroes exp(0)=1 there.
                    q0 = max(0, 128 * (m - 4 * c))
                    nc.tensor.matmul(
                        s2[:, i, q0:512],
                        lhsT=qkT[:, H_PER + h, m * 128:(m + 1) * 128],
                        rhs=qkT[:, h, c * 512 + q0:(c + 1) * 512],
                        start=True, stop=True,
                    )
                e = work.tile([128, 2, 512], BF16, tag="e", bufs=E_BUFS)
                if pr >= npr - 2:
                    # diagonal pair: exp + mask only the q-range below the
                    # diagonal per tile; cols below q0 are never read (the
                    # corresponding U-chain matmuls are skipped).
                    i0 = 2 * (pr - (npr - 2))
                    for i in range(2):
                        q0 = max(0, 128 * (2 * pr + i - 4 * c))
                        nc.scalar.activation(
                            e[:, i, q0:], s2[:, i, q0:],
                            mybir.ActivationFunctionType.Exp, scale=SCALE)
                        nc.gpsimd.tensor_mul(e[:, i, q0:], e[:, i, q0:],
                                             masks[:, i0 + i, q0:])
                else:
                    nc.scalar.activation(e[:], s2[:],
                                         mybir.ActivationFunctionType.Exp,
                                         scale=SCALE)
                e_pairs.append(e)
            pop(POP_AFTER if c > 0 else 1)

            # U chains per 128-query block: [q, j0..127 | G]
            yn = work.tile([128, 4, HEAD_DIM], BF16, tag="yn", bufs=YN_BUFS)
            for blk in range(4):
                u_ps = psum.tile([128, HEAD_DIM + 1], F32, tag="u", bufs=2)
                nmv = 4 * c + blk + 1   # key tiles visible to this q block
                for m in range(nmv):
                    nc.tensor.matmul(
                        u_ps[:],
                        lhsT=e_pairs[m // 2][:, m % 2, blk * 128:(blk + 1) * 128],
                        rhs=v_sb[:, m, h, :],
                        start=(m == 0), stop=(m == nmv - 1),
                    )
                gi = work.tile([128, 1], F32, tag="gi", bufs=GI_BUFS)
                nc.vector.reciprocal(gi[:], u_ps[:, HEAD_DIM:HEAD_DIM + 1])
                if NORM_ENGINE == "act":
                    nc.scalar.activation(
                        yn[:, blk, :], u_ps[:, 0:HEAD_DIM],
                        mybir.ActivationFunctionType.Copy, scale=gi[:])
                else:
                    nc.vector.tensor_scalar_mul(
                        yn[:, blk, :], u_ps[:, 0:HEAD_DIM], gi[:])
                pop(POP_BLK)
            # one XBAR DMA transpose: yn [q,(blk,j)] -> ynT [j,(blk,q)]
            nc.sync.dma_start(
                ynT[:, h, c * 512:(c + 1) * 512]
                    .rearrange("p (blk q) -> p blk q", blk=4),
                yn.rearrange("p blk j -> p (blk j)"),
                transpose=True,
            )

        st.copy_mix = False
        # queue outproj for this chunk; it pops inside later units
        for dm in range(D_MODEL // 128):
            st.filler.append(_make_outproj(nc, work, psum, st, wo_sb, ynT,
                                           outT, b, t0, dm, c))


def _make_outproj(nc, work, psum, st, wo_sb, ynT, outT, b, t0, dm, tcn):
    def thunk():
      with nc.named_scope(f"oproj.b{b}.c{tcn}"):
        ps = psum.tile([128, 512], F32, tag="pp", bufs=2)
        for kj in range(JL // 128):
            nc.tensor.matmul(
                ps[:],
                lhsT=wo_sb[:, kj, dm * 128:(dm + 1) * 128],
                rhs=ynT[:, kj, tcn * 512:(tcn + 1) * 512],
                start=(kj == 0), stop=(kj == JL // 128 - 1),
            )
        o_sb = work.tile([128, 512], BF16, tag="osb", bufs=OSB_BUFS)
        act_copy = (st.copy_mix and st.ocopy_i % 2 == 1) or (
            st.ocopy_i % OCOPY_MOD == OCOPY_MOD - 1)
        if act_copy:
            nc.scalar.copy(o_sb[:], ps[:])
        else:
            nc.vector.tensor_copy(o_sb[:], ps[:])
        st.ocopy_i += 1
        nc.sync.dma_start(
            outT[dm * 128:(dm + 1) * 128,
                 t0 + tcn * 512: t0 + (tcn + 1) * 512],
            o_sb[:],
        )
    return thunk


def make_in_maps(x, Wq, bq, Wk, bk, Wv, bv, Wo, bo):
    xT_np = np.ascontiguousarray(
        x.reshape(T, D_MODEL).T).astype(ml_dtypes.bfloat16)
    in_maps = []
    for c in range(N_CORES):
        sl = slice(c * JL, (c + 1) * JL)
        h0 = slice(c * JL, c * JL + 128)
        h1 = slice(c * JL + 128, c * JL + 256)
        wqk_np = np.concatenate(
            [Wq[:, h0], Wq[:, h1], Wk[:, h0], Wk[:, h1]],
            axis=1).astype(ml_dtypes.bfloat16)
        bqk_np = np.concatenate([bq[h0], bq[h1], bk[h0], bk[h1]]).astype(np.float32)
        wv_np = np.zeros((D_MODEL, JV), np.float32)
        bvb_np = np.zeros((JV,), np.float32)
        for h, hs in enumerate((h0, h1)):
            wv_np[:, h * 129:h * 129 + 128] = Wv[:, hs]
            bvb_np[h * 129:h * 129 + 128] = bv[hs]
            bvb_np[h * 129 + 128] = 1.0
        wo_np = np.ascontiguousarray(Wo[sl, :]).astype(ml_dtypes.bfloat16)
        in_maps.append({
            "xT": xT_np,
            "wqk": wqk_np,
            "bqk": bqk_np,
            "wv": wv_np.astype(ml_dtypes.bfloat16),
            "bvb": np.broadcast_to(bvb_np, (128, JV)).copy(),
            "wo": wo_np,
        })
    return in_maps


def kernel(x, Wq, bq, Wk, bk, Wv, bv, Wo, bo):
    global _CACHED_NC
    x, Wq, bq, Wk, bk, Wv, bv, Wo, bo = [
        np.asarray(a, np.float32) for a in (x, Wq, bq, Wk, bk, Wv, bv, Wo, bo)
    ]
    if _CACHED_NC is None:
        _CACHED_NC = build_program()
    nc = _CACHED_NC

    in_maps = make_in_maps(x, Wq, bq, Wk, bk, Wv, bv, Wo, bo)
    res = run_bass_kernel_spmd(nc, in_maps, core_ids=list(range(N_CORES)))

    acc = res.results[0]["outT"].astype(np.float32)
    for c in range(1, N_CORES):
        acc += res.results[c]["outT"].astype(np.float32)
    out = acc.T + bo[None, :]
    return np.ascontiguousarray(out.reshape(B, S, D_MODEL), dtype=np.float32)


# ---------------------------------------------------------------- dev tools

def _np_partial_reference(inputs, core):
    """fp32 numpy partial output for one core's heads (no bo)."""
    x = np.asarray(inputs["x"], np.float32).reshape(T, D_MODEL)
    sl = slice(core * JL, (core + 1) * JL)
    q = x @ np.asarray(inputs["Wq"])[:, sl] + np.asarray(inputs["bq"])[sl]
    k = x @ np.asarray(inputs["Wk"])[:, sl] + np.asarray(inputs["bk"])[sl]
    v = x @ np.asarray(inputs["Wv"])[:, sl] + np.asarray(inputs["bv"])[sl]
    y = np.zeros((T, JL), np.float32)
    for b in range(B):
        tb = slice(b * S, (b + 1) * S)
        for h in range(H_PER):
            js = slice(h * HEAD_DIM, (h + 1) * HEAD_DIM)
            qh, kh, vh = q[tb, js], k[tb, js], v[tb, js]
            s = (qh @ kh.T) * SCALE
            mask = np.triu(np.ones((S, S), bool), k=1)
            s[mask] = -np.inf
            s -= s.max(axis=1, keepdims=True)
            p = np.exp(s)
            p /= p.sum(axis=1, keepdims=True)
            y[tb, js] = p @ vh
    return (y @ np.asarray(inputs["Wo"])[sl, :]).T  # [D, T]


def _simulate_core0(trace_path=None):
    import reference
    from concourse.bass_interp import CoreSim

    inputs = {k: np.asarray(v) for k, v in reference.setup_inputs().items()}
    nc = build_program()
    in_map = make_in_maps(**inputs)[0]

    sim = CoreSim(nc, trace=trace_path is not None, publish_trace=False)
    for name, arr in in_map.items():
        sim.tensor(name)[:] = arr
    sim.simulate(check_with_hw=False)
    print(f"sim.time: {sim.time} ns")
    if trace_path and sim.perfetto is not None:
        open(trace_path, "wb").write(sim.perfetto.take_serialized())
        print(f"wrote {trace_path}")
    got = np.asarray(sim.tensor("outT"), np.float32)

    want = _np_partial_reference(inputs, 0)
    denom = np.abs(want).max()
    err = np.abs(got - want).max() / denom
    print(f"sim core0 partial: max={np.abs(got).max():.4f} "
          f"absmax_err={np.abs(got - want).max():.5f} rel={err:.5f}")


if __name__ == "__main__":
    import sys
    if "--sim" in sys.argv:
        tp = "/tmp/sim_trace.pftrace" if "--trace" in sys.argv else None
        _simulate_core0(tp)
    else:
        nc = build_program()
        n_inst = sum(len(bb.instructions) for bb in nc.m.functions[0].blocks)
        print(f"built: {n_inst} instructions")


# revision 34
# speedup vs baseline: 2.3975x; 2.3975x over previous
"""Multi-head self-attention (B=4, S=2048, D=2048, H=16, hd=128) on 8 trn2
NeuronCores.

Sharding: tensor-parallel over heads. Core c owns heads {2c, 2c+1}:
  - computes q/k/v projections for its 2 heads over all tokens,
  - runs causal attention for its (4 batches x 2 heads) units,
  - computes a partial output projection with its 256 rows of Wo.
Host sums the 8 partial (bf16) outputs and adds bo.

Single woven PE stream per batch (c-outer):
  proj chunk c -> attention units (c,h0),(c,h1) -> [outproj of chunk c-1
  interleaved as PE filler inside the ACT-paced unit pair loops].

Layouts:
  qT/kT: [j, t] from W-stationary matmuls (lhsT = W tile, rhs = xT tile).
  v:     [t, j] DIRECTLY from x-stationary matmuls (lhsT = xT tile,
         rhs = Wv tile) -- no PE transposes. Wv is augmented per head with
         a 129th column of zeros whose bias is 1.0, so each v key-tile
         slab [t, 129] carries an all-ones column.
  S^T:   [t_k, t_q] score tiles (lhsT = kT tile, rhs = qT chunk); exp on
         ACT (no max subtraction -- weights are 0.02-scaled, logits O(1));
         diagonal tiles get 0/1 masks multiplied after exp (on GPSIMD).
  U:     [t_q(128-block), 129] = accumulated (lhsT = e block, rhs = v slab)
         -- column 128 is the softmax denominator G for free (ones column
         of v). Normalize: DVE reciprocal of G [128,1], then ACT
         Copy-with-scale evacuates U*(1/G) -> yn [q, j] bf16.
  ynT:   one XBAR DMA transpose per unit turns yn [q, (blk, j)] into
         ynT [j, (blk, q)] -- no PSUM, no PE, no DVE.
  O^T:   [d', t] partial = (lhsT = Wo tile [j, d'], rhs = ynT chunk),
         stored to DRAM as bf16 (host sums partials in f32).

PSUM banks: s2 pairs 2x2 + U 2 + pp (proj/outproj chains) 2 = 8.
"""

import math
from collections import deque

import numpy as np
import ml_dtypes

import concourse.bass as bass
import concourse.bacc as bacc
import concourse.mybir as mybir
import concourse.tile as tile
from concourse.bass_utils import run_bass_kernel_spmd

BF16 = mybir.dt.bfloat16
F32 = mybir.dt.float32

B, S, D_MODEL = 4, 2048, 2048
N_HEADS, HEAD_DIM = 16, 128
N_CORES = 8
H_PER = N_HEADS // N_CORES          # 2 heads per core
JL = H_PER * HEAD_DIM               # 256 local j-columns
JV = H_PER * (HEAD_DIM + 1)         # 258 v-columns incl. per-head ones col
T = B * S                           # 8192 tokens
KD = D_MODEL // 128                 # 16 contraction tiles over d_model
TC = S // 512                       # 4 token chunks of 512 per batch
NQK = 2 * H_PER                     # 4 j-tiles for fused q+k
SCALE = 1.0 / math.sqrt(HEAD_DIM)

_CACHED_NC = None

# schedule-tuning knobs (sweepable)
E_BUFS = 9        # e-pair SBUF ring depth
POP_PAIR = 1      # filler pops per score pair
POP_AFTER = 3     # filler pops after the pair loop
POP_BLK = 1       # filler pops per U block
OSB_BUFS = 8      # o_sb ring depth
YN_BUFS = 2
GI_BUFS = 4
OCOPY_MOD = 1000  # 1-in-OCOPY_MOD o_sb copies go to ACT (1000 = all DVE)
NORM_ENGINE = "act"  # U*(1/G) evacuation engine: act | dve
COPY_MIX_C0 = False  # alternate o_sb copies DVE/ACT in c0 units


def build_program():
    nc = bacc.Bacc("TRN2", target_bir_lowering=False, debug=False)

    xT = nc.dram_tensor("xT", [D_MODEL, T], BF16, kind="ExternalInput").ap()
    wqk = nc.dram_tensor("wqk", [D_MODEL, NQK * 128], BF16, kind="ExternalInput").ap()
    bqk = nc.dram_tensor("bqk", [NQK * 128], F32, kind="ExternalInput").ap()
    wv = nc.dram_tensor("wv", [D_MODEL, JV], BF16, kind="ExternalInput").ap()
    bvb = nc.dram_tensor("bvb", [128, JV], F32, kind="ExternalInput").ap()
    wo = nc.dram_tensor("wo", [JL, D_MODEL], BF16, kind="ExternalInput").ap()
    outT = nc.dram_tensor("outT", [D_MODEL, T], BF16, kind="ExternalOutput").ap()

    xT_r = xT.rearrange("(k p) t -> p k t", p=128)        # [128, KD, T]

    with tile.TileContext(nc) as tc:
        with (
            tc.tile_pool(name="const", bufs=1) as const,
            tc.tile_pool(name="work", bufs=1) as work,
            tc.tile_pool(name="psum", bufs=1, space="PSUM") as psum,
        ):
            # ---- constants: only wqk k-chunk 0 + bqk lead the first xt ----
            wqk_sb = const.tile([128, KD, NQK * 128], BF16)
            wqk_r = wqk.rearrange("(k p) j -> p k j", p=128)
            nc.sync.dma_start(wqk_sb[:, 0:4, :], wqk_r[:, 0:4, :])
            bqk_sb = const.tile([128, NQK], F32)
            nc.sync.dma_start(bqk_sb[:], bqk.rearrange("(m p) -> p m", p=128))

            wv_sb = const.tile([128, KD, JV], BF16)
            bvb_sb = const.tile([128, JV], F32)
            wo_sb = const.tile([128, JL // 128, D_MODEL], BF16)

            def load_seg_consts(seg):
                # trailing consts after the last xt segment, in need order
                if seg < 3:
                    return
                wv_r = wv.rearrange("(k p) j -> p k j", p=128)
                for kc in range(1, 4):
                    nc.sync.dma_start(wqk_sb[:, 4 * kc:4 * (kc + 1), :],
                                      wqk_r[:, 4 * kc:4 * (kc + 1), :])
                    nc.sync.dma_start(wv_sb[:, 4 * (kc - 1):4 * kc, :],
                                      wv_r[:, 4 * (kc - 1):4 * kc, :])
                nc.sync.dma_start(wv_sb[:, 12:16, :], wv_r[:, 12:16, :])
                nc.sync.dma_start(bvb_sb[:], bvb[:, :])
                nc.sync.dma_start(
                    wo_sb[:], wo.rearrange("(k p) d -> p k d", p=128))

            # masks[i][r, u] = 1.0 if u >= 128*i + r else 0  (diagonal tiles)
            masks = const.tile([128, 4, 512], BF16)
            nc.gpsimd.memset(masks[:], 1.0)
            for i in range(4):
                nc.gpsimd.affine_select(
                    out=masks[:, i, :],
                    in_=masks[:, i, :],
                    compare_op=mybir.AluOpType.is_ge,
                    fill=0.0,
                    base=-128 * i,
                    pattern=[[1, 512]],
                    channel_multiplier=-1,
                )

            st = _State()
            for b in range(B):
                _emit_batch(nc, tc, work, psum, b,
                            xT_r, wqk_sb, bqk_sb, wv_sb, bvb_sb, wo_sb,
                            masks, outT, st,
                            post_first_xt=load_seg_consts if b == 0 else None)
            st.copy_mix = True
            while st.filler:
                st.filler.popleft()()

    nc.compile()
    return nc


class _State:
    def __init__(self):
        self.filler = deque()      # outproj-group thunks, ready to emit
        self.ocopy_i = 0           # o_sb evacuation round-robin counter
        self.copy_mix = False      # True where ACT has slack (c0 units, drain)


def _emit_batch(nc, tc, work, psum, b, xT_r, wqk_sb, bqk_sb, wv_sb, bvb_sb,
                wo_sb, masks, outT, st, post_first_xt=None):
    t0 = b * S

    qkT = work.tile([128, NQK, S], BF16, tag="qkT", bufs=2)
    v_sb = work.tile([128, S // 128, H_PER, HEAD_DIM + 1], BF16, tag="v", bufs=1)
    ynT = work.tile([128, H_PER, S], BF16, tag="ynT", bufs=2)

    def pop(n):
        for _ in range(n):
            if st.filler:
                st.filler.popleft()()

    for c in range(TC):
        # ---------------- projection chunk c ----------------
        with nc.named_scope(f"proj.b{b}.c{c}"):
            xt = work.tile([128, KD, 512], BF16, tag="xt", bufs=2)
            for seg in range(4):
                nc.sync.dma_start(
                    xt[:, 4 * seg:4 * (seg + 1), :],
                    xT_r[:, 4 * seg:4 * (seg + 1), t0 + c * 512: t0 + (c + 1) * 512])
                if post_first_xt is not None:
                    post_first_xt(seg)
            post_first_xt = None
            # q/k chains (W-stationary): qkT[j, t]
            def qk_bias(jm, ps):
                nc.vector.tensor_scalar_add(
                    qkT[:, jm, c * 512:(c + 1) * 512], ps[:],
                    bqk_sb[:, jm:jm + 1],
                )

            if b == 0 and c == 0:
                # first chunk: ping-pong two chains across the arriving
                # wqk/xt k-segments so PE starts before all DMAs land
                ps01 = [psum.tile([128, 512], F32, tag="pp", bufs=2,
                                  name=f"ps0{i}")
                        for i in range(2)]
                for kc in range(4):
                    for jm in range(2):
                        for k in range(4 * kc, 4 * kc + 4):
                            nc.tensor.matmul(
                                ps01[jm][:],
                                lhsT=wqk_sb[:, k, jm * 128:(jm + 1) * 128],
                                rhs=xt[:, k, :],
                                start=(k == 0), stop=(k == KD - 1),
                            )
                for jm in range(2):
                    qk_bias(jm, ps01[jm])
                jm_rest = range(2, NQK)
            else:
                jm_rest = range(NQK)
            for jm in jm_rest:
                ps = psum.tile([128, 512], F32, tag="pp", bufs=2)
                for k in range(KD):
                    nc.tensor.matmul(
                        ps[:],
                        lhsT=wqk_sb[:, k, jm * 128:(jm + 1) * 128],
                        rhs=xt[:, k, :],
                        start=(k == 0), stop=(k == KD - 1),
                    )
                qk_bias(jm, ps)
            # v chains (x-stationary): v[t, j] slabs with ones columns
            for blk in range(4):
                m = 4 * c + blk
                pv = psum.tile([128, JV], F32, tag="pp", bufs=2)
                for k in range(KD):
                    nc.tensor.matmul(
                        pv[:],
                        lhsT=xt[:, k, blk * 128:(blk + 1) * 128],
                        rhs=wv_sb[:, k, :],
                        start=(k == 0), stop=(k == KD - 1),
                    )
                nc.vector.tensor_add(
                    v_sb[:, m, :, :],
                    pv.rearrange("p (h j) -> p h j", h=H_PER),
                    bvb_sb.rearrange("p (h j) -> p h j", h=H_PER),
                )

        # ---------------- attention units (c, h) ----------------
        nm = 4 * (c + 1)            # valid 128-wide key tiles
        npr = nm // 2
        st.copy_mix = COPY_MIX_C0 and (c == 0)
        for h in range(H_PER):
          with nc.named_scope(f"att.b{b}.c{c}.h{h}"):
            e_pairs = []
            for pr in range(npr):
                pop(POP_PAIR)
                s2 = psum.tile([128, 2, 512], F32, tag="s2", bufs=2)
                for i in range(2):
                    m = 2 * pr + i
                    # causal: tile m only sees queries q >= 128*(m-4c);
                    # the unwritten psum region reads 0 (start zeroes the
                    # bank) and the mask multiply zeroes exp(0)=1 there.
                    q0 = max(0, 128 * (m - 4 * c))
                    nc.tensor.matmul(
                        s2[:, i, q0:512],
                        lhsT=qkT[:, H_PER + h, m * 128:(m + 1) * 128],
                        rhs=qkT[:, h, c * 512 + q0:(c + 1) * 512],
                        start=True, stop=True,
                    )
                e = work.tile([128, 2, 512], BF16, tag="e", bufs=E_BUFS)
                if pr >= npr - 2:
                    # diagonal pair: exp + mask only the q-range below the
                    # diagonal per tile; cols below q0 are never read (the
                    # corresponding U-chain matmuls are skipped).
                    i0 = 2 * (pr - (npr - 2))
                    for i in range(2):
                        q0 = max(0, 128 * (2 * pr + i - 4 * c))
                        nc.scalar.activation(
                            e[:, i, q0:], s2[:, i, q0:],
                            mybir.ActivationFunctionType.Exp, scale=SCALE)
                        nc.gpsimd.tensor_mul(e[:, i, q0:], e[:, i, q0:],
                                             masks[:, i0 + i, q0:])
                else:
                    nc.scalar.activation(e[:], s2[:],
                                         mybir.ActivationFunctionType.Exp,
                                         scale=SCALE)
                e_pairs.append(e)
            pop(POP_AFTER if c > 0 else 1)

            # U chains per 128-query block: [q, j0..127 | G]
            yn = work.tile([128, 4, HEAD_DIM], BF16, tag="yn", bufs=YN_BUFS)
            for blk in range(4):
                u_ps = psum.tile([128, HEAD_DIM + 1], F32, tag="u", bufs=2)
                nmv = 4 * c + blk + 1   # key tiles visible to this q block
                for m in range(nmv):
                    nc.tensor.matmul(
                        u_ps[:],
                        lhsT=e_pairs[m // 2][:, m % 2, blk * 128:(blk + 1) * 128],
                        rhs=v_sb[:, m, h, :],
                        start=(m == 0), stop=(m == nmv - 1),
                    )
                gi = work.tile([128, 1], F32, tag="gi", bufs=GI_BUFS)
                nc.vector.reciprocal(gi[:], u_ps[:, HEAD_DIM:HEAD_DIM + 1])
                if NORM_ENGINE == "act":
                    nc.scalar.activation(
                        yn[:, blk, :], u_ps[:, 0:HEAD_DIM],
                        mybir.ActivationFunctionType.Copy, scale=gi[:])
                else:
                    nc.vector.tensor_scalar_mul(
                        yn[:, blk, :], u_ps[:, 0:HEAD_DIM], gi[:])
                pop(POP_BLK)
            # one XBAR DMA transpose: yn [q,(blk,j)] -> ynT [j,(blk,q)]
            nc.sync.dma_start(
                ynT[:, h, c * 512:(c + 1) * 512]
                    .rearrange("p (blk q) -> p blk q", blk=4),
                yn.rearrange("p blk j -> p (blk j)"),
                transpose=True,
            )

        st.copy_mix = False
        # queue outproj for this chunk; it pops inside later units
        for dm in range(D_MODEL // 128):
            st.filler.append(_make_outproj(nc, work, psum, st, wo_sb, ynT,
                                           outT, b, t0, dm, c))


def _make_outproj(nc, work, psum, st, wo_sb, ynT, outT, b, t0, dm, tcn):
    def thunk():
      with nc.named_scope(f"oproj.b{b}.c{tcn}"):
        ps = psum.tile([128, 512], F32, tag="pp", bufs=2)
        for kj in range(JL // 128):
            nc.tensor.matmul(
                ps[:],
                lhsT=wo_sb[:, kj, dm * 128:(dm + 1) * 128],
                rhs=ynT[:, kj, tcn * 512:(tcn + 1) * 512],
                start=(kj == 0), stop=(kj == JL // 128 - 1),
            )
        o_sb = work.tile([128, 512], BF16, tag="osb", bufs=OSB_BUFS)
        act_copy = (st.copy_mix and st.ocopy_i % 2 == 1) or (
            st.ocopy_i % OCOPY_MOD == OCOPY_MOD - 1)
        if act_copy:
            nc.scalar.copy(o_sb[:], ps[:])
        else:
            nc.vector.tensor_copy(o_sb[:], ps[:])
        st.ocopy_i += 1
        nc.sync.dma_start(
            outT[dm * 128:(dm + 1) * 128,
                 t0 + tcn * 512: t0 + (tcn + 1) * 512],
            o_sb[:],
        )
    return thunk


def make_in_maps(x, Wq, bq, Wk, bk, Wv, bv, Wo, bo):
    xT_np = np.ascontiguousarray(
        x.reshape(T, D_MODEL).T).astype(ml_dtypes.bfloat16)
    in_maps = []
    for c in range(N_CORES):
        sl = slice(c * JL, (c + 1) * JL)
        h0 = slice(c * JL, c * JL + 128)
        h1 = slice(c * JL + 128, c * JL + 256)
        wqk_np = np.concatenate(
            [Wq[:, h0], Wq[:, h1], Wk[:, h0], Wk[:, h1]],
            axis=1).astype(ml_dtypes.bfloat16)
        bqk_np = np.concatenate([bq[h0], bq[h1], bk[h0], bk[h1]]).astype(np.float32)
        wv_np = np.zeros((D_MODEL, JV), np.float32)
        bvb_np = np.zeros((JV,), np.float32)
        for h, hs in enumerate((h0, h1)):
            wv_np[:, h * 129:h * 129 + 128] = Wv[:, hs]
            bvb_np[h * 129:h * 129 + 128] = bv[hs]
            bvb_np[h * 129 + 128] = 1.0
        wo_np = np.ascontiguousarray(Wo[sl, :]).astype(ml_dtypes.bfloat16)
        in_maps.append({
            "xT": xT_np,
            "wqk": wqk_np,
            "bqk": bqk_np,
            "wv": wv_np.astype(ml_dtypes.bfloat16),
            "bvb": np.broadcast_to(bvb_np, (128, JV)).copy(),
            "wo": wo_np,
        })
    return in_maps


def kernel(x, Wq, bq, Wk, bk, Wv, bv, Wo, bo):
    global _CACHED_NC
    x, Wq, bq, Wk, bk, Wv, bv, Wo, bo = [
        np.asarray(a, np.float32) for a in (x, Wq, bq, Wk, bk, Wv, bv, Wo, bo)
    ]
    if _CACHED_NC is None:
        _CACHED_NC = build_program()
    nc = _CACHED_NC

    in_maps = make_in_maps(x, Wq, bq, Wk, bk, Wv, bv, Wo, bo)
    res = run_bass_kernel_spmd(nc, in_maps, core_ids=list(range(N_CORES)))

    acc = res.results[0]["outT"].astype(np.float32)
    for c in range(1, N_CORES):
        acc += res.results[c]["outT"].astype(np.float32)
    out = acc.T + bo[None, :]
    return np.ascontiguousarray(out.reshape(B, S, D_MODEL), dtype=np.float32)


# ---------------------------------------------------------------- dev tools

def _np_partial_reference(inputs, core):
    """fp32 numpy partial output for one core's heads (no bo)."""
    x = np.asarray(inputs["x"], np.float32).reshape(T, D_MODEL)
    sl = slice(core * JL, (core + 1) * JL)
    q = x @ np.asarray(inputs["Wq"])[:, sl] + np.asarray(inputs["bq"])[sl]
    k = x @ np.asarray(inputs["Wk"])[:, sl] + np.asarray(inputs["bk"])[sl]
    v = x @ np.asarray(inputs["Wv"])[:, sl] + np.asarray(inputs["bv"])[sl]
    y = np.zeros((T, JL), np.float32)
    for b in range(B):
        tb = slice(b * S, (b + 1) * S)
        for h in range(H_PER):
            js = slice(h * HEAD_DIM, (h + 1) * HEAD_DIM)
            qh, kh, vh = q[tb, js], k[tb, js], v[tb, js]
            s = (qh @ kh.T) * SCALE
            mask = np.triu(np.ones((S, S), bool), k=1)
            s[mask] = -np.inf
            s -= s.max(axis=1, keepdims=True)
            p = np.exp(s)
            p /= p.sum(axis=1, keepdims=True)
            y[tb, js] = p @ vh
    return (y @ np.asarray(inputs["Wo"])[sl, :]).T  # [D, T]


def _simulate_core0(trace_path=None):
    import reference
    from concourse.bass_interp import CoreSim

    inputs = {k: np.asarray(v) for k, v in reference.setup_inputs().items()}
    nc = build_program()
    in_map = make_in_maps(**inputs)[0]

    sim = CoreSim(nc, trace=trace_path is not None, publish_trace=False)
    for name, arr in in_map.items():
        sim.tensor(name)[:] = arr
    sim.simulate(check_with_hw=False)
    print(f"sim.time: {sim.time} ns")
    if trace_path and sim.perfetto is not None:
        open(trace_path, "wb").write(sim.perfetto.take_serialized())
        print(f"wrote {trace_path}")
    got = np.asarray(sim.tensor("outT"), np.float32)

    want = _np_partial_reference(inputs, 0)
    denom = np.abs(want).max()
    err = np.abs(got - want).max() / denom
    print(f"sim core0 partial: max={np.abs(got).max():.4f} "
          f"absmax_err={np.abs(got - want).max():.5f} rel={err:.5f}")


if __name__ == "__main__":
    import sys
    if "--sim" in sys.argv:
        tp = "/tmp/sim_trace.pftrace" if "--trace" in sys.argv else None
        _simulate_core0(tp)
    else:
        nc = build_program()
        n_inst = sum(len(bb.instructions) for bb in nc.m.functions[0].blocks)
        print(f"built: {n_inst} instructions")
